# revision 7
# baseline (speedup 1.0000x reference)
"""AttnPool Trainium2 kernel (bf16-stream version).

Math: the reference computes k = z @ W.T, scores = (q . k)/sqrt(D),
attn = softmax(scores over P), out = attn-weighted sum of z. Since q is a
single query vector, q.(z@W.T) == z.(q@W): precompute qw = q @ W (tiny, host),
then the device kernel is one memory-bound pass over z.

Precision: z and qw are cast to bf16 on the host — softmax attn weights and
an attn-weighted mean tolerate it easily (measured end-to-end rel err ~3e-3
vs the 2e-2 gate) — halving both the HBM stream (24 -> 12 MiB/core) and the
SBUF traffic, and unlocking the DVE 2x packed mode for part of the scores.

Per batch (8 per core, SPMD over 8 cores), at [128, 768] row-tile grain:
  sync HWDGE: z chunks [128, 2, 768] into per-chunk SBUF tiles (independent
              tags — same-tile chunk writes would serialize on WAW)
  scores:     5 tiles fused on DVE (scalar_tensor_tensor+accum, 1x mode) and
              3 tiles as plain 2x tensor_tensor products reduced on ACT
              (activation Copy + accum_out, scale folds in 1/sqrt(D)) —
              the fused op never engages the 2x uop, so splitting across
              DVE+ACT beats 8 fused dots
  ACT exp:    one exp over the batch's [128, 8] score columns -> e (bf16)
  PE:         pooled acc += e_t.T @ z_t per tile, two PSUM half-banks;
              S = ones.T @ e, ACT accum -> S, DVE reciprocal -> 1/S,
              output row normalized during the PSUM->SBUF copy (ACT,
              scale=1/S) one batch late so no engine stream stalls.

Measured (For_i slope, device-resident inputs): ~62 us/core vs ~81 us for
the f32 version of the same pipeline; DMA floor alone is ~54 us (the bf16
stream sustains only ~290 GB/s/core vs f32's ~360 — unexplained, see
work/ benches), PE's z readback costs ~8 us of span on top, and the
remaining ~8 us is the DVE dot chain sticking out above the stream.

Sharding: data-parallel over batch, 8 batches per core on 8 cores (SPMD).
"""
import os

os.environ.setdefault("NEURON_RT_RESET_CORES", "1")

import numpy as np

import concourse.tile as tile
from concourse import bacc, mybir
from concourse.bass_utils import run_bass_kernel_spmd

B, P, D = 64, 1024, 768
N_CORES = 8
B_PER_CORE = B // N_CORES
P_TILES = P // 128
SCALE = float(1.0 / np.sqrt(np.float32(D)))
HALF = D // 2

f32 = mybir.dt.float32
f32r = mybir.dt.float32r
bf16 = mybir.dt.bfloat16

_cache = {}


def make_pools(tc):
    return (
        tc.tile_pool(name="consts", bufs=1),
        tc.tile_pool(name="zp", bufs=3),
        tc.tile_pool(name="sc", bufs=2),
        tc.tile_pool(name="scr", bufs=3),
        tc.tile_pool(name="ps", bufs=2, space="PSUM"),
    )


def emit_consts(nc, consts, qw_dram):
    # qw broadcast rides SWDGE so it never delays the z stream (HWDGE)
    qw_bc = consts.tile([128, D], f32, name="qw_bc")
    nc.gpsimd.dma_start(out=qw_bc[:], in_=qw_dram.to_broadcast((128, D)))
    ones_col = consts.tile([128, 1], f32, name="ones_col")
    nc.vector.memset(ones_col[:], 1.0)
    junk_row = consts.tile([1, P_TILES], f32, name="junk_row")
    return qw_bc, ones_col, junk_row


def emit_body(
    nc,
    tc,
    pools,
    consts,
    z_dram,
    out_dram,
    dma_tiles=2,
    dma_engines=("sync",),
    dot_cols=D,
):
    """One full pass over this core's 8 batches, incl. all 8 output rows.

    dma_engines: engines whose HWDGE queues the z-stream DMAs rotate over.
    dot_cols: timing-diagnostic only — restrict the score dot-product to the
    first dot_cols features (wrong math unless == D; isolates DVE load).
    """
    _, zp, scp, scrp, psp = pools
    qw_bc, ones_col, junk_row = consts
    dma_i = 0

    def emit_out(prev):
        b_prev, pool_prev0, pool_prev1, S_prev = prev
        rS = scp.tile([1, 1], f32, name="rS", tag="rS")
        nc.vector.reciprocal(rS[:], S_prev[0:1, 0:1])
        out_row = scp.tile([1, D], f32, name="out_row", tag="out_row")
        for h, pps in enumerate([pool_prev0, pool_prev1]):
            nc.scalar.activation(
                out=out_row[0:1, h * HALF : (h + 1) * HALF],
                in_=pps[:],
                func=mybir.ActivationFunctionType.Copy,
                scale=rS[0:1, 0:1],
            )
        nc.scalar.dma_start(out=out_dram[b_prev : b_prev + 1, :], in_=out_row[:])

    prev = None
    for b in range(B_PER_CORE):
        z_sb = zp.tile([128, P_TILES, D], f32r, name="z_sb", tag="z_sb")
        s_buf = scp.tile([128, P_TILES], f32, name="s_buf", tag="s_buf")
        e_buf = scp.tile([128, P_TILES], f32r, name="e_buf", tag="e_buf")
        pool_ps0 = psp.tile([1, HALF], f32, name="pool_ps0", tag="pool_ps0")
        pool_ps1 = psp.tile([1, HALF], f32, name="pool_ps1", tag="pool_ps1")

        for t in range(P_TILES):
            if t % dma_tiles == 0:
                eng = getattr(nc, dma_engines[dma_i % len(dma_engines)])
                dma_i += 1
                eng.dma_start(
                    out=z_sb[:, t : t + dma_tiles, :],
                    in_=z_dram[b, t * 128 : (t + dma_tiles) * 128, :]
                    .rearrange("(g p) d -> p g d", p=128)
                    .bitcast(f32r),
                )
            scratch = scrp.tile([128, D], f32, name="scratch", tag="scratch")
            nc.vector.scalar_tensor_tensor(
                out=scratch[:, :dot_cols],
                in0=z_sb[:, t, :dot_cols].bitcast(f32),
                scalar=SCALE,
                in1=qw_bc[:, :dot_cols],
                op0=mybir.AluOpType.mult,
                op1=mybir.AluOpType.mult,
                accum_out=s_buf[:, t : t + 1],
            )
            nc.scalar.activation(
                out=e_buf[:, t : t + 1],
                in_=s_buf[:, t : t + 1],
                func=mybir.ActivationFunctionType.Exp,
            )
            for h, pps in enumerate([pool_ps0, pool_ps1]):
                nc.tensor.matmul(
                    out=pps[:],
                    lhsT=e_buf[:, t : t + 1],
                    rhs=z_sb[:, t, h * HALF : (h + 1) * HALF],
                    start=(t == 0),
                    stop=(t == P_TILES - 1),
                )

        # softmax denominator: S = sum over all 1024 e values
        S_row = psp.tile([1, P_TILES], f32, name="S_row", tag="S_row")
        nc.tensor.matmul(
            out=S_row[:], lhsT=ones_col[:], rhs=e_buf[:].bitcast(f32), start=True, stop=True
        )
        S_val = scp.tile([1, 1], f32, name="S_val", tag="S_val")
        nc.scalar.activation(
            out=junk_row[:],
            in_=S_row[:],
            func=mybir.ActivationFunctionType.Copy,
            accum_out=S_val[:],
        )

        if prev is not None:
            emit_out(prev)
        prev = (b, pool_ps0, pool_ps1, S_val)

    emit_out(prev)


def emit_consts_bf16(nc, consts, qw_dram):
    qw_bc = consts.tile([128, D], bf16, name="qw_bc")
    nc.gpsimd.dma_start(out=qw_bc[:], in_=qw_dram.to_broadcast((128, D)))
    ones_col = consts.tile([128, 1], bf16, name="ones_col")
    nc.vector.memset(ones_col[:], 1.0)
    junk_row = consts.tile([1, P_TILES], f32, name="junk_row")
    return qw_bc, ones_col, junk_row


def emit_body_bf16(
    nc,
    tc,
    pools,
    consts,
    z_dram,
    out_dram,
    dma_tiles=2,
    dma_engines=("sync",),
    row_pairs=False,
    token_major=False,
    dot_op="stt",
    dot_split=None,
    dot_cols=D,
    chunk_tags=False,
    pe_lite=False,
    dma_only=False,
):
    """bf16-stream variant: z/qw arrive bf16 (host-cast), scores via DVE 2x
    bf16 mode, one exp per batch, pooled PE matmuls in bf16 with f32 PSUM.

    dma_only: timing-diagnostic only — emit just the z-stream DMAs plus one
    dependent out-row write (wrong math; isolates the DMA floor).

    row_pairs: map two consecutive tokens onto each SBUF partition row so
    every DMA descriptor covers 3072B contiguous (token = 256*st + 2*p + two;
    softmax/pooling are token-permutation invariant so output is unchanged).
    token_major: map dma_tiles consecutive tokens onto each partition row
    (token = chunk_base + p*dma_tiles + g) so each partition's DMA run is
    dma_tiles*1536B contiguous and the whole chunk is one contiguous DRAM
    span; with dma_tiles=8 each batch is a single fully-contiguous 1.57 MB
    descriptor set (12 KB/partition). Same permutation-invariance argument.
    dot_op: "stt" (scalar_tensor_tensor) or "ttr" (tensor_tensor_reduce).
    dot_split: None, or (n_fused, n_dve_prod, n_gp_prod) summing to P_TILES —
    per batch, the first n_gp_prod tiles' score products run on GPSIMD and
    the next n_dve_prod on DVE as plain 2x tensor_tensor (both reduced on
    ACT via accum-copy); the last n_fused run fused 1x on DVE. Balances the
    score dot across DVE/ACT/GPSIMD instead of serializing it all on DVE.
    """
    _, zp, scp, scrp, psp = pools
    qw_bc, ones_col, junk_row = consts
    dma_i = 0

    def scratch_alloc(pool):
        return pool.tile([128, D], bf16, name="scratch", tag="scratch")

    def emit_out(prev):
        b_prev, pool_prev0, pool_prev1, S_prev = prev
        rS = scp.tile([1, 1], f32, name="rS", tag="rS")
        nc.vector.reciprocal(rS[:], S_prev[0:1, 0:1])
        out_row = scp.tile([1, D], f32, name="out_row", tag="out_row")
        for h, pps in enumerate([pool_prev0, pool_prev1]):
            nc.scalar.activation(
                out=out_row[0:1, h * HALF : (h + 1) * HALF],
                in_=pps[:],
                func=mybir.ActivationFunctionType.Copy,
                scale=rS[0:1, 0:1],
            )
        nc.scalar.dma_start(out=out_dram[b_prev : b_prev + 1, :], in_=out_row[:])

    prev = None
    for b in range(B_PER_CORE):
        if chunk_tags:
            # independent tile (tag) per DMA chunk: no same-tile WAW ordering
            # between chunks, so multiple queues can genuinely overlap
            n_chunks = P_TILES // dma_tiles
            z_chunks = [
                zp.tile(
                    [128, dma_tiles, D], bf16, name=f"zc{ci}", tag=f"zc{ci}"
                )
                for ci in range(n_chunks)
            ]

            def z_tile(t):
                return z_chunks[t // dma_tiles][:, t % dma_tiles, :]

        else:
            z_sb = zp.tile([128, P_TILES, D], bf16, name="z_sb", tag="z_sb")

            def z_tile(t):
                return z_sb[:, t, :]

        s_buf = scp.tile([128, P_TILES], f32, name="s_buf", tag="s_buf")
        e_buf = scp.tile([128, P_TILES], bf16, name="e_buf", tag="e_buf")
        pool_ps0 = psp.tile([1, HALF], f32, name="pool_ps0", tag="pool_ps0")
        pool_ps1 = psp.tile([1, HALF], f32, name="pool_ps1", tag="pool_ps1")

        for t in range(P_TILES):
            if t % dma_tiles == 0:
                eng = getattr(nc, dma_engines[dma_i % len(dma_engines)])
                dma_i += 1
                if token_major:
                    src = z_dram[b, t * 128 : (t + dma_tiles) * 128, :].rearrange(
                        "(p g) d -> p g d", g=dma_tiles
                    )
                elif row_pairs:
                    # token = slab*256 + 2*p + two -> columns (2u, 2u+1) hold a
                    # 256-token slab; per-partition runs are 2 tokens = 3072B
                    assert dma_tiles % 2 == 0
                    src = z_dram[b, t * 128 : (t + dma_tiles) * 128, :].rearrange(
                        "(g p two) d -> p (g two) d", p=128, two=2
                    )
                else:
                    src = z_dram[b, t * 128 : (t + dma_tiles) * 128, :].rearrange(
                        "(g p) d -> p g d", p=128
                    )
                if chunk_tags:
                    dst = z_chunks[t // dma_tiles][:, :, :]
                else:
                    dst = z_sb[:, t : t + dma_tiles, :]
                eng.dma_start(out=dst, in_=src)
            if dma_only:
                continue
            if dot_split is not None:
                split_b = (
                    dot_split[b % len(dot_split)]
                    if isinstance(dot_split[0], tuple)
                    else dot_split
                )
                nf, nd, ng = split_b
                on_gp = t < ng
                on_dve_prod = ng <= t < ng + nd
            else:
                on_gp = on_dve_prod = False
            if on_gp or on_dve_prod:
                # 2x-mode product (plain tensor_tensor), reduced on ACT with
                # the softmax scale folded into the reduce.
                tag = "gp_prod" if on_gp else "dv_prod"
                prod = scrp.tile([128, D], bf16, name=tag, tag=tag)
                eng2 = nc.gpsimd if on_gp else nc.vector
                eng2.tensor_tensor(
                    out=prod[:],
                    in0=z_tile(t),
                    in1=qw_bc[:],
                    op=mybir.AluOpType.mult,
                )
                ared = scrp.tile([128, D], bf16, name="ared", tag="ared")
                nc.scalar.activation(
                    out=ared[:],
                    in_=prod[:],
                    func=mybir.ActivationFunctionType.Copy,
                    scale=SCALE,
                    accum_out=s_buf[:, t : t + 1],
                )
            elif dot_op == "ttr":
                nc.vector.tensor_tensor_reduce(
                    out=scratch_alloc(scrp)[:],
                    in0=z_tile(t),
                    in1=qw_bc[:],
                    scale=SCALE,
                    scalar=0.0,
                    op0=mybir.AluOpType.mult,
                    op1=mybir.AluOpType.add,
                    accum_out=s_buf[:, t : t + 1],
                )
            else:
                scratch = scrp.tile([128, D], bf16, name="scratch", tag="scratch")
                nc.vector.scalar_tensor_tensor(
                    out=scratch[:, :dot_cols],
                    in0=z_tile(t)[:, :dot_cols],
                    scalar=SCALE,
                    in1=qw_bc[:, :dot_cols],
                    op0=mybir.AluOpType.mult,
                    op1=mybir.AluOpType.mult,
                    accum_out=s_buf[:, t : t + 1],
                )

        if dma_only:
            if b == B_PER_CORE - 1:
                out_row = scp.tile([1, D], f32, name="out_row", tag="out_row")
                nc.scalar.activation(
                    out=out_row[:],
                    in_=z_tile(P_TILES - 1)[0:1, :],
                    func=mybir.ActivationFunctionType.Copy,
                )
                nc.scalar.dma_start(out=out_dram[0:1, :], in_=out_row[:])
            continue

        # one exp per batch over all 8 score columns
        nc.scalar.activation(
            out=e_buf[:],
            in_=s_buf[:],
            func=mybir.ActivationFunctionType.Exp,
        )
        for t in range(P_TILES):
            if pe_lite and t > 0:
                continue  # timing-diagnostic only: 1/8th of PE matmuls
            for h, pps in enumerate([pool_ps0, pool_ps1]):
                nc.tensor.matmul(
                    out=pps[:],
                    lhsT=e_buf[:, t : t + 1],
                    rhs=z_tile(t)[:, h * HALF : (h + 1) * HALF],
                    start=(t == 0),
                    stop=(t == P_TILES - 1) or pe_lite,
                )

        S_row = psp.tile([1, P_TILES], f32, name="S_row", tag="S_row")
        nc.tensor.matmul(
            out=S_row[:], lhsT=ones_col[:], rhs=e_buf[:], start=True, stop=True
        )
        S_val = scp.tile([1, 1], f32, name="S_val", tag="S_val")
        nc.scalar.activation(
            out=junk_row[:],
            in_=S_row[:],
            func=mybir.ActivationFunctionType.Copy,
            accum_out=S_val[:],
        )

        if prev is not None:
            emit_out(prev)
        prev = (b, pool_ps0, pool_ps1, S_val)

    if prev is not None:
        emit_out(prev)


def build_bf16(reps=1, dma_tiles=2, zp_bufs=4, **kw):
    nc = bacc.Bacc("TRN2", target_bir_lowering=False, debug=False, num_devices=N_CORES)
    z_dram = nc.dram_tensor("z", [B_PER_CORE, P, D], bf16, kind="ExternalInput").ap()
    qw_dram = nc.dram_tensor("qw", [1, D], bf16, kind="ExternalInput").ap()
    out_dram = nc.dram_tensor("out", [B_PER_CORE, D], f32, kind="ExternalOutput").ap()

    with tile.TileContext(nc) as tc:
        with (
            tc.tile_pool(name="consts", bufs=1) as consts,
            tc.tile_pool(name="zp", bufs=zp_bufs) as zp,
            tc.tile_pool(name="sc", bufs=2) as scp,
            tc.tile_pool(name="scr", bufs=3) as scrp,
            tc.tile_pool(name="ps", bufs=2, space="PSUM") as psp,
        ):
            pools = (consts, zp, scp, scrp, psp)
            ck = emit_consts_bf16(nc, consts, qw_dram)
            for rep in range(reps):
                emit_body_bf16(
                    nc, tc, pools, ck, z_dram, out_dram, dma_tiles=dma_tiles, **kw
                )

    nc.finalize()
    return nc


def build(reps=1, dma_tiles=2):
    nc = bacc.Bacc("TRN2", target_bir_lowering=False, debug=False, num_devices=N_CORES)
    z_dram = nc.dram_tensor("z", [B_PER_CORE, P, D], f32, kind="ExternalInput").ap()
    qw_dram = nc.dram_tensor("qw", [1, D], f32, kind="ExternalInput").ap()
    out_dram = nc.dram_tensor("out", [B_PER_CORE, D], f32, kind="ExternalOutput").ap()

    with tile.TileContext(nc) as tc:
        pools_cm = make_pools(tc)
        with (
            pools_cm[0] as consts,
            pools_cm[1] as zp,
            pools_cm[2] as scp,
            pools_cm[3] as scrp,
            pools_cm[4] as psp,
        ):
            pools = (consts, zp, scp, scrp, psp)
            ck = emit_consts(nc, consts, qw_dram)
            for rep in range(reps):
                emit_body(nc, tc, pools, ck, z_dram, out_dram, dma_tiles=dma_tiles)

    nc.finalize()
    return nc


def get_nc(reps=1, dma_tiles=2):
    key = (reps, dma_tiles)
    if key not in _cache:
        _cache[key] = build(reps, dma_tiles)
    return _cache[key]


# Final tuned configuration (see work/ benches): bf16 stream, independent
# per-chunk tiles (no same-tile WAW serialization between chunk DMAs), and
# per batch 5 fused DVE dots + 3 DVE 2x-mode products reduced on ACT.
BEST_KW = dict(
    dma_tiles=2,
    chunk_tags=True,
    dot_split=(5, 3, 0),
)


def get_nc_bf16(reps=1, **kw):
    merged = {**BEST_KW, **kw}
    key = ("bf16", reps, tuple(sorted(merged.items())))
    if key not in _cache:
        _cache[key] = build_bf16(reps, **merged)
    return _cache[key]


def run(z, qw, reps=1, **kwargs):
    """Run the SPMD kernel. z: [B,P,D] f32, qw: [D] f32. Returns results obj."""
    nc = get_nc(reps)
    in_maps = [
        {"z": z[i * B_PER_CORE : (i + 1) * B_PER_CORE], "qw": qw[None, :]}
        for i in range(N_CORES)
    ]
    return run_bass_kernel_spmd(nc, in_maps, core_ids=list(range(N_CORES)), **kwargs)


def run_bf16(zb, qwb, reps=1, build_kw=None, **kwargs):
    """zb: [B,P,D] bf16, qwb: [D] bf16 (pre-scaled not required)."""
    nc = get_nc_bf16(reps, **(build_kw or {}))
    in_maps = [
        {"z": zb[i * B_PER_CORE : (i + 1) * B_PER_CORE], "qw": qwb[None, :]}
        for i in range(N_CORES)
    ]
    return run_bass_kernel_spmd(nc, in_maps, core_ids=list(range(N_CORES)), **kwargs)


def kernel(z, q, W_proj):
    import ml_dtypes

    z = np.asarray(z, dtype=np.float32)
    q = np.asarray(q, dtype=np.float32)
    W_proj = np.asarray(W_proj, dtype=np.float32)
    qw = (q.reshape(D) @ W_proj).astype(np.float32)

    zb = np.ascontiguousarray(z.astype(ml_dtypes.bfloat16))
    qwb = qw.astype(ml_dtypes.bfloat16)

    res = run_bf16(zb, qwb)
    out = np.concatenate([r["out"] for r in res.results], axis=0)
    return out.astype(np.float32)


if __name__ == "__main__":
    rng = np.random.default_rng(0)
    z = rng.standard_normal((B, P, D)).astype(np.float32)
    q = rng.standard_normal((1, 1, D)).astype(np.float32)
    W = (rng.standard_normal((D, D)) / np.sqrt(D)).astype(np.float32)
    out = kernel(z, q, W)
    print("out", out.shape, out.dtype, out[:2, :4])



# revision 8
# speedup vs baseline: 1.1650x; 1.1650x over previous
"""AttnPool Trainium2 kernel (bf16-stream version).

Math: the reference computes k = z @ W.T, scores = (q . k)/sqrt(D),
attn = softmax(scores over P), out = attn-weighted sum of z. Since q is a
single query vector, q.(z@W.T) == z.(q@W): precompute qw = q @ W (tiny, host),
then the device kernel is one memory-bound pass over z.

Precision: z and qw are cast to bf16 on the host — softmax attn weights and
an attn-weighted mean tolerate it easily (measured end-to-end rel err ~3e-3
vs the 2e-2 gate) — halving both the HBM stream (24 -> 12 MiB/core) and the
SBUF traffic, and unlocking the DVE 2x packed mode for part of the scores.

Per batch (8 per core, SPMD over 8 cores), at [128, 768] row-tile grain:
  sync HWDGE: z chunks [128, 2, 768] into per-chunk SBUF tiles (independent
              tags — same-tile chunk writes would serialize on WAW)
  scores:     5 tiles fused on DVE (scalar_tensor_tensor+accum, 1x mode) and
              3 tiles as plain 2x tensor_tensor products reduced on ACT
              (activation Copy + accum_out, scale folds in 1/sqrt(D)) —
              the fused op never engages the 2x uop, so splitting across
              DVE+ACT beats 8 fused dots
  ACT exp:    one exp over the batch's [128, 8] score columns -> e (bf16)
  PE:         pooled acc += e_t.T @ z_t per tile, two PSUM half-banks;
              S = ones.T @ e, ACT accum -> S, DVE reciprocal -> 1/S,
              output row normalized during the PSUM->SBUF copy (ACT,
              scale=1/S) one batch late so no engine stream stalls.

Measured (For_i slope, device-resident inputs): ~62 us/core vs ~81 us for
the f32 version of the same pipeline; DMA floor alone is ~54 us (the bf16
stream sustains only ~290 GB/s/core vs f32's ~360 — unexplained, see
work/ benches), PE's z readback costs ~8 us of span on top, and the
remaining ~8 us is the DVE dot chain sticking out above the stream.

Sharding: data-parallel over batch, 8 batches per core on 8 cores (SPMD).
"""
import os

os.environ.setdefault("NEURON_RT_RESET_CORES", "1")

import numpy as np

import concourse.tile as tile
from concourse import bacc, mybir
from concourse.bass_utils import run_bass_kernel_spmd

B, P, D = 64, 1024, 768
N_CORES = 8
B_PER_CORE = B // N_CORES
P_TILES = P // 128
SCALE = float(1.0 / np.sqrt(np.float32(D)))
HALF = D // 2

f32 = mybir.dt.float32
f32r = mybir.dt.float32r
bf16 = mybir.dt.bfloat16

_cache = {}


def make_pools(tc):
    return (
        tc.tile_pool(name="consts", bufs=1),
        tc.tile_pool(name="zp", bufs=3),
        tc.tile_pool(name="sc", bufs=2),
        tc.tile_pool(name="scr", bufs=3),
        tc.tile_pool(name="ps", bufs=2, space="PSUM"),
    )


def emit_consts(nc, consts, qw_dram):
    # qw broadcast rides SWDGE so it never delays the z stream (HWDGE)
    qw_bc = consts.tile([128, D], f32, name="qw_bc")
    nc.gpsimd.dma_start(out=qw_bc[:], in_=qw_dram.to_broadcast((128, D)))
    ones_col = consts.tile([128, 1], f32, name="ones_col")
    nc.vector.memset(ones_col[:], 1.0)
    junk_row = consts.tile([1, P_TILES], f32, name="junk_row")
    return qw_bc, ones_col, junk_row


def emit_body(
    nc,
    tc,
    pools,
    consts,
    z_dram,
    out_dram,
    dma_tiles=2,
    dma_engines=("sync",),
    dot_cols=D,
):
    """One full pass over this core's 8 batches, incl. all 8 output rows.

    dma_engines: engines whose HWDGE queues the z-stream DMAs rotate over.
    dot_cols: timing-diagnostic only — restrict the score dot-product to the
    first dot_cols features (wrong math unless == D; isolates DVE load).
    """
    _, zp, scp, scrp, psp = pools
    qw_bc, ones_col, junk_row = consts
    dma_i = 0

    def emit_out(prev):
        b_prev, pool_prev0, pool_prev1, S_prev = prev
        rS = scp.tile([1, 1], f32, name="rS", tag="rS")
        nc.vector.reciprocal(rS[:], S_prev[0:1, 0:1])
        out_row = scp.tile([1, D], f32, name="out_row", tag="out_row")
        for h, pps in enumerate([pool_prev0, pool_prev1]):
            nc.scalar.activation(
                out=out_row[0:1, h * HALF : (h + 1) * HALF],
                in_=pps[:],
                func=mybir.ActivationFunctionType.Copy,
                scale=rS[0:1, 0:1],
            )
        nc.scalar.dma_start(out=out_dram[b_prev : b_prev + 1, :], in_=out_row[:])

    prev = None
    for b in range(B_PER_CORE):
        z_sb = zp.tile([128, P_TILES, D], f32r, name="z_sb", tag="z_sb")
        s_buf = scp.tile([128, P_TILES], f32, name="s_buf", tag="s_buf")
        e_buf = scp.tile([128, P_TILES], f32r, name="e_buf", tag="e_buf")
        pool_ps0 = psp.tile([1, HALF], f32, name="pool_ps0", tag="pool_ps0")
        pool_ps1 = psp.tile([1, HALF], f32, name="pool_ps1", tag="pool_ps1")

        for t in range(P_TILES):
            if t % dma_tiles == 0:
                eng = getattr(nc, dma_engines[dma_i % len(dma_engines)])
                dma_i += 1
                eng.dma_start(
                    out=z_sb[:, t : t + dma_tiles, :],
                    in_=z_dram[b, t * 128 : (t + dma_tiles) * 128, :]
                    .rearrange("(g p) d -> p g d", p=128)
                    .bitcast(f32r),
                )
            scratch = scrp.tile([128, D], f32, name="scratch", tag="scratch")
            nc.vector.scalar_tensor_tensor(
                out=scratch[:, :dot_cols],
                in0=z_sb[:, t, :dot_cols].bitcast(f32),
                scalar=SCALE,
                in1=qw_bc[:, :dot_cols],
                op0=mybir.AluOpType.mult,
                op1=mybir.AluOpType.mult,
                accum_out=s_buf[:, t : t + 1],
            )
            nc.scalar.activation(
                out=e_buf[:, t : t + 1],
                in_=s_buf[:, t : t + 1],
                func=mybir.ActivationFunctionType.Exp,
            )
            for h, pps in enumerate([pool_ps0, pool_ps1]):
                nc.tensor.matmul(
                    out=pps[:],
                    lhsT=e_buf[:, t : t + 1],
                    rhs=z_sb[:, t, h * HALF : (h + 1) * HALF],
                    start=(t == 0),
                    stop=(t == P_TILES - 1),
                )

        # softmax denominator: S = sum over all 1024 e values
        S_row = psp.tile([1, P_TILES], f32, name="S_row", tag="S_row")
        nc.tensor.matmul(
            out=S_row[:], lhsT=ones_col[:], rhs=e_buf[:].bitcast(f32), start=True, stop=True
        )
        S_val = scp.tile([1, 1], f32, name="S_val", tag="S_val")
        nc.scalar.activation(
            out=junk_row[:],
            in_=S_row[:],
            func=mybir.ActivationFunctionType.Copy,
            accum_out=S_val[:],
        )

        if prev is not None:
            emit_out(prev)
        prev = (b, pool_ps0, pool_ps1, S_val)

    emit_out(prev)


def emit_consts_bf16(nc, consts, qw_dram):
    qw_bc = consts.tile([128, D], bf16, name="qw_bc")
    nc.gpsimd.dma_start(out=qw_bc[:], in_=qw_dram.to_broadcast((128, D)))
    ones_col = consts.tile([128, 1], bf16, name="ones_col")
    nc.vector.memset(ones_col[:], 1.0)
    junk_row = consts.tile([1, P_TILES], f32, name="junk_row")
    return qw_bc, ones_col, junk_row


def emit_body_bf16(
    nc,
    tc,
    pools,
    consts,
    z_dram,
    out_dram,
    dma_tiles=2,
    dma_engines=("sync",),
    row_pairs=False,
    token_major=False,
    dot_op="stt",
    dot_split=None,
    dot_cols=D,
    chunk_tags=False,
    pe_lite=False,
    dma_only=False,
):
    """bf16-stream variant: z/qw arrive bf16 (host-cast), scores via DVE 2x
    bf16 mode, one exp per batch, pooled PE matmuls in bf16 with f32 PSUM.

    dma_only: timing-diagnostic only — emit just the z-stream DMAs plus one
    dependent out-row write (wrong math; isolates the DMA floor).

    row_pairs: map two consecutive tokens onto each SBUF partition row so
    every DMA descriptor covers 3072B contiguous (token = 256*st + 2*p + two;
    softmax/pooling are token-permutation invariant so output is unchanged).
    token_major: map dma_tiles consecutive tokens onto each partition row
    (token = chunk_base + p*dma_tiles + g) so each partition's DMA run is
    dma_tiles*1536B contiguous and the whole chunk is one contiguous DRAM
    span; with dma_tiles=8 each batch is a single fully-contiguous 1.57 MB
    descriptor set (12 KB/partition). Same permutation-invariance argument.
    dot_op: "stt" (scalar_tensor_tensor) or "ttr" (tensor_tensor_reduce).
    dot_split: None, or (n_fused, n_dve_prod, n_gp_prod) summing to P_TILES —
    per batch, the first n_gp_prod tiles' score products run on GPSIMD and
    the next n_dve_prod on DVE as plain 2x tensor_tensor (both reduced on
    ACT via accum-copy); the last n_fused run fused 1x on DVE. Balances the
    score dot across DVE/ACT/GPSIMD instead of serializing it all on DVE.
    """
    _, zp, scp, scrp, psp = pools
    qw_bc, ones_col, junk_row = consts
    dma_i = 0

    def scratch_alloc(pool):
        return pool.tile([128, D], bf16, name="scratch", tag="scratch")

    def emit_out(prev):
        b_prev, pool_prev0, pool_prev1, S_prev = prev
        rS = scp.tile([1, 1], f32, name="rS", tag="rS")
        nc.vector.reciprocal(rS[:], S_prev[0:1, 0:1])
        out_row = scp.tile([1, D], f32, name="out_row", tag="out_row")
        for h, pps in enumerate([pool_prev0, pool_prev1]):
            nc.scalar.activation(
                out=out_row[0:1, h * HALF : (h + 1) * HALF],
                in_=pps[:],
                func=mybir.ActivationFunctionType.Copy,
                scale=rS[0:1, 0:1],
            )
        nc.scalar.dma_start(out=out_dram[b_prev : b_prev + 1, :], in_=out_row[:])

    prev = None
    for b in range(B_PER_CORE):
        if chunk_tags:
            # independent tile (tag) per DMA chunk: no same-tile WAW ordering
            # between chunks, so multiple queues can genuinely overlap
            n_chunks = P_TILES // dma_tiles
            z_chunks = [
                zp.tile(
                    [128, dma_tiles, D], bf16, name=f"zc{ci}", tag=f"zc{ci}"
                )
                for ci in range(n_chunks)
            ]

            def z_tile(t):
                return z_chunks[t // dma_tiles][:, t % dma_tiles, :]

        else:
            z_sb = zp.tile([128, P_TILES, D], bf16, name="z_sb", tag="z_sb")

            def z_tile(t):
                return z_sb[:, t, :]

        s_buf = scp.tile([128, P_TILES], f32, name="s_buf", tag="s_buf")
        e_buf = scp.tile([128, P_TILES], bf16, name="e_buf", tag="e_buf")
        pool_ps0 = psp.tile([1, HALF], f32, name="pool_ps0", tag="pool_ps0")
        pool_ps1 = psp.tile([1, HALF], f32, name="pool_ps1", tag="pool_ps1")

        for t in range(P_TILES):
            if t % dma_tiles == 0:
                eng = getattr(nc, dma_engines[dma_i % len(dma_engines)])
                dma_i += 1
                if token_major:
                    src = z_dram[b, t * 128 : (t + dma_tiles) * 128, :].rearrange(
                        "(p g) d -> p g d", g=dma_tiles
                    )
                elif row_pairs:
                    # token = slab*256 + 2*p + two -> columns (2u, 2u+1) hold a
                    # 256-token slab; per-partition runs are 2 tokens = 3072B
                    assert dma_tiles % 2 == 0
                    src = z_dram[b, t * 128 : (t + dma_tiles) * 128, :].rearrange(
                        "(g p two) d -> p (g two) d", p=128, two=2
                    )
                else:
                    src = z_dram[b, t * 128 : (t + dma_tiles) * 128, :].rearrange(
                        "(g p) d -> p g d", p=128
                    )
                if chunk_tags:
                    dst = z_chunks[t // dma_tiles][:, :, :]
                else:
                    dst = z_sb[:, t : t + dma_tiles, :]
                eng.dma_start(out=dst, in_=src)
            if dma_only:
                continue
            if dot_split is not None:
                split_b = (
                    dot_split[b % len(dot_split)]
                    if isinstance(dot_split[0], tuple)
                    else dot_split
                )
                nf, nd, ng = split_b
                on_gp = t < ng
                on_dve_prod = ng <= t < ng + nd
            else:
                on_gp = on_dve_prod = False
            if on_gp or on_dve_prod:
                # 2x-mode product (plain tensor_tensor), reduced on ACT with
                # the softmax scale folded into the reduce.
                tag = "gp_prod" if on_gp else "dv_prod"
                prod = scrp.tile([128, D], bf16, name=tag, tag=tag)
                eng2 = nc.gpsimd if on_gp else nc.vector
                eng2.tensor_tensor(
                    out=prod[:],
                    in0=z_tile(t),
                    in1=qw_bc[:],
                    op=mybir.AluOpType.mult,
                )
                ared = scrp.tile([128, D], bf16, name="ared", tag="ared")
                nc.scalar.activation(
                    out=ared[:],
                    in_=prod[:],
                    func=mybir.ActivationFunctionType.Copy,
                    scale=SCALE,
                    accum_out=s_buf[:, t : t + 1],
                )
            elif dot_op == "ttr":
                nc.vector.tensor_tensor_reduce(
                    out=scratch_alloc(scrp)[:],
                    in0=z_tile(t),
                    in1=qw_bc[:],
                    scale=SCALE,
                    scalar=0.0,
                    op0=mybir.AluOpType.mult,
                    op1=mybir.AluOpType.add,
                    accum_out=s_buf[:, t : t + 1],
                )
            else:
                scratch = scrp.tile([128, D], bf16, name="scratch", tag="scratch")
                nc.vector.scalar_tensor_tensor(
                    out=scratch[:, :dot_cols],
                    in0=z_tile(t)[:, :dot_cols],
                    scalar=SCALE,
                    in1=qw_bc[:, :dot_cols],
                    op0=mybir.AluOpType.mult,
                    op1=mybir.AluOpType.mult,
                    accum_out=s_buf[:, t : t + 1],
                )

        if dma_only:
            if b == B_PER_CORE - 1:
                out_row = scp.tile([1, D], f32, name="out_row", tag="out_row")
                nc.scalar.activation(
                    out=out_row[:],
                    in_=z_tile(P_TILES - 1)[0:1, :],
                    func=mybir.ActivationFunctionType.Copy,
                )
                nc.scalar.dma_start(out=out_dram[0:1, :], in_=out_row[:])
            continue

        # one exp per batch over all 8 score columns
        nc.scalar.activation(
            out=e_buf[:],
            in_=s_buf[:],
            func=mybir.ActivationFunctionType.Exp,
        )
        for t in range(P_TILES):
            if pe_lite and t > 0:
                continue  # timing-diagnostic only: 1/8th of PE matmuls
            for h, pps in enumerate([pool_ps0, pool_ps1]):
                nc.tensor.matmul(
                    out=pps[:],
                    lhsT=e_buf[:, t : t + 1],
                    rhs=z_tile(t)[:, h * HALF : (h + 1) * HALF],
                    start=(t == 0),
                    stop=(t == P_TILES - 1) or pe_lite,
                )

        S_row = psp.tile([1, P_TILES], f32, name="S_row", tag="S_row")
        nc.tensor.matmul(
            out=S_row[:], lhsT=ones_col[:], rhs=e_buf[:], start=True, stop=True
        )
        S_val = scp.tile([1, 1], f32, name="S_val", tag="S_val")
        nc.scalar.activation(
            out=junk_row[:],
            in_=S_row[:],
            func=mybir.ActivationFunctionType.Copy,
            accum_out=S_val[:],
        )

        if prev is not None:
            emit_out(prev)
        prev = (b, pool_ps0, pool_ps1, S_val)

    if prev is not None:
        emit_out(prev)


def emit_body_v2(
    nc,
    tc,
    pools,
    consts,
    z_dram,
    out_dram,
    dma_tiles=4,
    dma_engines=("sync",),
    dot_plan=(("F", "F", "F", "PA", "PA", "PA", "GA", "PG"),),
    out_engs=("act", "dve"),
):
    """v2: token-major contiguous z stream, flavor-scheduled score dots,
    softmax scale folded into the exp, and host-side normalization (the
    kernel ships pooled halves + the 8 S partial sums per batch; the host
    divides). Removes the reciprocal/S_val chain entirely.

    dot_plan: per-batch tuple (cycled) of P_TILES flavor strings:
      F  = fused mult+accum on DVE (1x)
      PA = DVE 2x product, ACT accum-copy reduce
      GA = GPSIMD product, ACT accum-copy reduce
      PG = DVE 2x product, GPSIMD tensor_reduce
      GD = GPSIMD product, DVE tensor_reduce (2x)
      GF = fused mult+accum on GPSIMD
    out_engs: engines for the two pooled-half PSUM->SBUF copies.
    """
    _, zp, scp, scrp, psp = pools
    qw_bc, ones_col = consts[0], consts[1]
    dma_i = 0
    n_chunks = P_TILES // dma_tiles

    def emit_out(prev):
        b_prev, pps_list, S_prev = prev
        out_row = scp.tile([1, D + P_TILES], f32, name="out_row", tag="out_row")
        for h, pps in enumerate(pps_list):
            dst = out_row[0:1, h * HALF : (h + 1) * HALF]
            if out_engs[h] == "act":
                nc.scalar.activation(
                    out=dst, in_=pps[:], func=mybir.ActivationFunctionType.Copy
                )
            else:
                nc.vector.tensor_scalar(
                    out=dst,
                    in0=pps[:],
                    scalar1=1.0,
                    scalar2=None,
                    op0=mybir.AluOpType.mult,
                )
        nc.vector.tensor_scalar(
            out=out_row[0:1, D : D + P_TILES],
            in0=S_prev[:],
            scalar1=1.0,
            scalar2=None,
            op0=mybir.AluOpType.mult,
        )
        nc.scalar.dma_start(out=out_dram[b_prev : b_prev + 1, :], in_=out_row[:])

    prev = None
    for b in range(B_PER_CORE):
        z_chunks = [
            zp.tile([128, dma_tiles, D], bf16, name=f"zc{ci}", tag=f"zc{ci}")
            for ci in range(n_chunks)
        ]

        def z_tile(t):
            return z_chunks[t // dma_tiles][:, t % dma_tiles, :]

        s_buf = scp.tile([128, P_TILES], f32, name="s_buf", tag="s_buf")
        e_buf = scp.tile([128, P_TILES], bf16, name="e_buf", tag="e_buf")
        pool_ps0 = psp.tile([1, HALF], f32, name="pool_ps0", tag="pool_ps0")
        pool_ps1 = psp.tile([1, HALF], f32, name="pool_ps1", tag="pool_ps1")

        plan = dot_plan[b % len(dot_plan)]
        assert len(plan) == P_TILES
        for t in range(P_TILES):
            if t % dma_tiles == 0:
                eng = getattr(nc, dma_engines[dma_i % len(dma_engines)])
                dma_i += 1
                eng.dma_start(
                    out=z_chunks[t // dma_tiles][:, :, :],
                    in_=z_dram[b, t * 128 : (t + dma_tiles) * 128, :].rearrange(
                        "(p g) d -> p g d", g=dma_tiles
                    ),
                )
            fl = plan[t]
            if fl == "F" or fl == "GF":
                eng2 = nc.vector if fl == "F" else nc.gpsimd
                scratch = scrp.tile([128, D], bf16, name="sF", tag="sF")
                eng2.scalar_tensor_tensor(
                    out=scratch[:],
                    in0=z_tile(t),
                    scalar=1.0,
                    in1=qw_bc[:],
                    op0=mybir.AluOpType.mult,
                    op1=mybir.AluOpType.mult,
                    accum_out=s_buf[:, t : t + 1],
                )
                continue
            p_eng = nc.gpsimd if fl in ("GA", "GD") else nc.vector
            tag = "prodG" if fl in ("GA", "GD") else "prodP"
            prod = scrp.tile([128, D], bf16, name=tag, tag=tag)
            p_eng.tensor_tensor(
                out=prod[:], in0=z_tile(t), in1=qw_bc[:], op=mybir.AluOpType.mult
            )
            if fl in ("PA", "GA"):
                ared = scrp.tile([128, D], bf16, name="ared", tag="ared")
                nc.scalar.activation(
                    out=ared[:],
                    in_=prod[:],
                    func=mybir.ActivationFunctionType.Copy,
                    accum_out=s_buf[:, t : t + 1],
                )
            else:
                r_eng = nc.gpsimd if fl == "PG" else nc.vector
                r_eng.tensor_reduce(
                    out=s_buf[:, t : t + 1],
                    in_=prod[:],
                    axis=mybir.AxisListType.X,
                    op=mybir.AluOpType.add,
                )

        # exp with the softmax scale folded in (dots are unscaled)
        nc.scalar.activation(
            out=e_buf[:],
            in_=s_buf[:],
            func=mybir.ActivationFunctionType.Exp,
            scale=SCALE,
        )
        for t in range(P_TILES):
            for h, pps in enumerate([pool_ps0, pool_ps1]):
                nc.tensor.matmul(
                    out=pps[:],
                    lhsT=e_buf[:, t : t + 1],
                    rhs=z_tile(t)[:, h * HALF : (h + 1) * HALF],
                    start=(t == 0),
                    stop=(t == P_TILES - 1),
                )
        S_row = psp.tile([1, P_TILES], f32, name="S_row", tag="S_row")
        nc.tensor.matmul(
            out=S_row[:], lhsT=ones_col[:], rhs=e_buf[:], start=True, stop=True
        )

        if prev is not None:
            emit_out(prev)
        prev = (b, [pool_ps0, pool_ps1], S_row)

    if prev is not None:
        emit_out(prev)


def build_v2(reps=1, zp_bufs=4, **kw):
    nc = bacc.Bacc("TRN2", target_bir_lowering=False, debug=False, num_devices=N_CORES)
    z_dram = nc.dram_tensor("z", [B_PER_CORE, P, D], bf16, kind="ExternalInput").ap()
    qw_dram = nc.dram_tensor("qw", [1, D], bf16, kind="ExternalInput").ap()
    out_dram = nc.dram_tensor(
        "out", [B_PER_CORE, D + P_TILES], f32, kind="ExternalOutput"
    ).ap()

    with tile.TileContext(nc) as tc:
        with (
            tc.tile_pool(name="consts", bufs=1) as consts,
            tc.tile_pool(name="zp", bufs=zp_bufs) as zp,
            tc.tile_pool(name="sc", bufs=2) as scp,
            tc.tile_pool(name="scr", bufs=3) as scrp,
            tc.tile_pool(name="ps", bufs=2, space="PSUM") as psp,
        ):
            pools = (consts, zp, scp, scrp, psp)
            ck = emit_consts_bf16(nc, consts, qw_dram)
            for rep in range(reps):
                emit_body_v2(nc, tc, pools, ck, z_dram, out_dram, **kw)

    nc.finalize()
    return nc


def build_bf16(reps=1, dma_tiles=2, zp_bufs=4, **kw):
    nc = bacc.Bacc("TRN2", target_bir_lowering=False, debug=False, num_devices=N_CORES)
    z_dram = nc.dram_tensor("z", [B_PER_CORE, P, D], bf16, kind="ExternalInput").ap()
    qw_dram = nc.dram_tensor("qw", [1, D], bf16, kind="ExternalInput").ap()
    out_dram = nc.dram_tensor("out", [B_PER_CORE, D], f32, kind="ExternalOutput").ap()

    with tile.TileContext(nc) as tc:
        with (
            tc.tile_pool(name="consts", bufs=1) as consts,
            tc.tile_pool(name="zp", bufs=zp_bufs) as zp,
            tc.tile_pool(name="sc", bufs=2) as scp,
            tc.tile_pool(name="scr", bufs=3) as scrp,
            tc.tile_pool(name="ps", bufs=2, space="PSUM") as psp,
        ):
            pools = (consts, zp, scp, scrp, psp)
            ck = emit_consts_bf16(nc, consts, qw_dram)
            for rep in range(reps):
                emit_body_bf16(
                    nc, tc, pools, ck, z_dram, out_dram, dma_tiles=dma_tiles, **kw
                )

    nc.finalize()
    return nc


def build(reps=1, dma_tiles=2):
    nc = bacc.Bacc("TRN2", target_bir_lowering=False, debug=False, num_devices=N_CORES)
    z_dram = nc.dram_tensor("z", [B_PER_CORE, P, D], f32, kind="ExternalInput").ap()
    qw_dram = nc.dram_tensor("qw", [1, D], f32, kind="ExternalInput").ap()
    out_dram = nc.dram_tensor("out", [B_PER_CORE, D], f32, kind="ExternalOutput").ap()

    with tile.TileContext(nc) as tc:
        pools_cm = make_pools(tc)
        with (
            pools_cm[0] as consts,
            pools_cm[1] as zp,
            pools_cm[2] as scp,
            pools_cm[3] as scrp,
            pools_cm[4] as psp,
        ):
            pools = (consts, zp, scp, scrp, psp)
            ck = emit_consts(nc, consts, qw_dram)
            for rep in range(reps):
                emit_body(nc, tc, pools, ck, z_dram, out_dram, dma_tiles=dma_tiles)

    nc.finalize()
    return nc


def get_nc(reps=1, dma_tiles=2):
    key = (reps, dma_tiles)
    if key not in _cache:
        _cache[key] = build(reps, dma_tiles)
    return _cache[key]


# Final tuned configuration (see work/ benches): bf16 stream, independent
# per-chunk tiles (no same-tile WAW serialization between chunk DMAs), and
# per batch 5 fused DVE dots + 3 DVE 2x-mode products reduced on ACT.
BEST_KW = dict(
    dma_tiles=2,
    chunk_tags=True,
    dot_split=(5, 3, 0),
)


def get_nc_bf16(reps=1, **kw):
    merged = {**BEST_KW, **kw}
    key = ("bf16", reps, tuple(sorted(merged.items())))
    if key not in _cache:
        _cache[key] = build_bf16(reps, **merged)
    return _cache[key]


def run(z, qw, reps=1, **kwargs):
    """Run the SPMD kernel. z: [B,P,D] f32, qw: [D] f32. Returns results obj."""
    nc = get_nc(reps)
    in_maps = [
        {"z": z[i * B_PER_CORE : (i + 1) * B_PER_CORE], "qw": qw[None, :]}
        for i in range(N_CORES)
    ]
    return run_bass_kernel_spmd(nc, in_maps, core_ids=list(range(N_CORES)), **kwargs)


def run_bf16(zb, qwb, reps=1, build_kw=None, **kwargs):
    """zb: [B,P,D] bf16, qwb: [D] bf16 (pre-scaled not required)."""
    nc = get_nc_bf16(reps, **(build_kw or {}))
    in_maps = [
        {"z": zb[i * B_PER_CORE : (i + 1) * B_PER_CORE], "qw": qwb[None, :]}
        for i in range(N_CORES)
    ]
    return run_bass_kernel_spmd(nc, in_maps, core_ids=list(range(N_CORES)), **kwargs)


def kernel(z, q, W_proj):
    import ml_dtypes

    z = np.asarray(z, dtype=np.float32)
    q = np.asarray(q, dtype=np.float32)
    W_proj = np.asarray(W_proj, dtype=np.float32)
    qw = (q.reshape(D) @ W_proj).astype(np.float32)

    zb = np.ascontiguousarray(z.astype(ml_dtypes.bfloat16))
    qwb = qw.astype(ml_dtypes.bfloat16)

    res = run_bf16(zb, qwb)
    out = np.concatenate([r["out"] for r in res.results], axis=0)
    return out.astype(np.float32)


if __name__ == "__main__":
    rng = np.random.default_rng(0)
    z = rng.standard_normal((B, P, D)).astype(np.float32)
    q = rng.standard_normal((1, 1, D)).astype(np.float32)
    W = (rng.standard_normal((D, D)) / np.sqrt(D)).astype(np.float32)
    out = kernel(z, q, W)
    print("out", out.shape, out.dtype, out[:2, :4])

